# revision 12
# baseline (speedup 1.0000x reference)
"""LlamaTernaryMLP (SwiGLU MLP with ternary weights + per-channel scales) on 8 TRN2 cores.

Strategy: pure data-parallel over the 8192 tokens (1024 tokens/core, no
collectives).  Each core runs the full MLP on its token shard with all three
weight matrices streamed from HBM in bf16 (ternary values are exact in bf16;
only the activations lose precision, fp32 accumulation in PSUM).

Layout (host-prepped so every DMA is >=1KB-contiguous per partition):
  activations are kept feature-major on chip ([feature,token]); weights are
  pre-transposed/tiled so each matmul is lhsT=[K=128, M=128] stationary weight
  tile against a moving [K=128, N=512] activation tile.

Per core, per 512-token block:
  gate/up: for each of 86 I-tiles accumulate 32 K-tiles over HIDDEN into two
  PSUM banks, then h = silu(gate*ag) * (up*au) -> bf16 SBUF (86 tiles resident).
  down:    for each of 32 H-tiles accumulate 86 K-tiles over INTER, scale by ad,
  DMA out fp32.
"""

import numpy as np
import ml_dtypes

import concourse.bass as bass
import concourse.mybir as mybir
import concourse.tile as tile
from concourse import bacc
from concourse.bass_utils import run_bass_kernel_spmd

P = 128
B, S, HID, INT = 4, 2048, 4096, 11008
NCORES = 8
BLK = 512
CS = 128  # matmul column-strip width (128 = no column tiling; 32/64 tested slower)
# Weight dtype: ternary {-1,0,+1} is exact in fp8e4m3.  fp8 stationary halves
# the FWL weight-load time (4 values per 32-bit read vs 2 for bf16) and the
# weight DMA traffic; the moving activations stay bf16 (mixed-dtype matmul).
WDT = "f8"  # "f8" | "bf16"
SIM_AF = None  # set to "sigmoid" in CoreSim tests (Silu not implemented there)


def _af(AF):
    return AF.Sigmoid if SIM_AF == "sigmoid" else AF.Silu


def build_nc(t_loc, hid, inter, blk=BLK, reps=1):
    """Build the per-core Bass program for a t_loc-token shard.

    reps>1 wraps the whole computation in a hardware For_i loop (identical
    compute each iteration) — benchmarking only, so per-iteration time can be
    extracted from wall-clock above the axon RPC floor.
    """
    nblk = t_loc // blk
    kt = hid // P    # K-tiles over hidden (gate/up contraction)
    it = inter // P  # I-tiles (intermediate channels / down contraction)
    ht = hid // P    # output H-tiles
    bf16 = mybir.dt.bfloat16
    f32 = mybir.dt.float32
    wdt = mybir.dt.float8e4 if WDT == "f8" else bf16
    AF = mybir.ActivationFunctionType
    OP = mybir.AluOpType

    nc = bacc.Bacc(
        "TRN2", target_bir_lowering=False, debug=False, num_devices=NCORES
    )
    xp = nc.declare_dram_parameter("xp", [nblk, P, kt, blk], bf16, isOutput=False)
    wgp = nc.declare_dram_parameter("wgp", [it, P, kt, P], wdt, isOutput=False)
    wup = nc.declare_dram_parameter("wup", [it, P, kt, P], wdt, isOutput=False)
    wdp = nc.declare_dram_parameter("wdp", [ht, P, it, P], wdt, isOutput=False)
    ags = nc.declare_dram_parameter("ags", [P, it], f32, isOutput=False)
    aus = nc.declare_dram_parameter("aus", [P, it], f32, isOutput=False)
    ads = nc.declare_dram_parameter("ads", [P, ht], f32, isOutput=False)
    outp = nc.declare_dram_parameter("outp", [nblk, ht, P, blk], f32, isOutput=True)

    half = (it + 1) // 2  # down-proj weight strips stream in two halves

    with tile.TileContext(nc) as tc:
        with (
            tc.tile_pool(name="consts", bufs=1) as cpool,
            tc.tile_pool(name="xpool", bufs=1) as xpool,
            tc.tile_pool(name="wpool", bufs=2) as wpool,
            tc.tile_pool(name="wdpool", bufs=2) as wdpool,
            tc.tile_pool(name="hpool", bufs=it) as hpool,
            tc.tile_pool(name="epool", bufs=2) as epool,
            tc.tile_pool(name="opool", bufs=2) as opool,
            tc.tile_pool(name="psg", bufs=2, space=bass.MemorySpace.PSUM) as psg,
            tc.tile_pool(name="psu", bufs=2, space=bass.MemorySpace.PSUM) as psu,
            tc.tile_pool(name="pso", bufs=2, space=bass.MemorySpace.PSUM) as pso,
        ):
            ag_sb = cpool.tile([P, it], f32, tag="ag")
            au_sb = cpool.tile([P, it], f32, tag="au")
            ad_sb = cpool.tile([P, ht], f32, tag="ad")
            nc.sync.dma_start(ag_sb[:], ags[:])
            nc.sync.dma_start(au_sb[:], aus[:])
            nc.sync.dma_start(ad_sb[:], ads[:])

            def body():
                _build_body(
                    nc, tc, nblk, kt, it, ht, blk,
                    xp, wgp, wup, wdp, outp,
                    ag_sb, au_sb, ad_sb,
                    xpool, wpool, wdpool, hpool, epool, opool, psg, psu, pso,
                    half, bf16, f32, AF, OP, wdt,
                )

            if reps == 1:
                body()
            else:
                with tc.For_i(0, reps, 1):
                    body()
    nc.compile()
    return nc


def _build_body(
    nc, tc, nblk, kt, it, ht, blk,
    xp, wgp, wup, wdp, outp,
    ag_sb, au_sb, ad_sb,
    xpool, wpool, wdpool, hpool, epool, opool, psg, psu, pso,
    half, bf16, f32, AF, OP, wdt,
):
    for b in range(nblk):
                x_sb = xpool.tile([P, kt, blk], bf16, tag="x")
                nc.sync.dma_start(x_sb[:], xp[b])

                h_tiles = []
                for i in range(it):
                    wg_sb = wpool.tile([P, kt, P], wdt, tag="wg")
                    wu_sb = wpool.tile([P, kt, P], wdt, tag="wu")
                    nc.sync.dma_start(wg_sb[:], wgp[i])
                    nc.sync.dma_start(wu_sb[:], wup[i])
                    g_ps = psg.tile([P, blk], f32, tag="g")
                    u_ps = psu.tile([P, blk], f32, tag="u")
                    # CS=128: plain full-array matmuls. Column tiling
                    # (CS=32/64) was measured slower — the per-weight-change
                    # serialization is the array drain, which strips also pay.
                    for n in range(kt):
                        for c in range(0, P, CS):
                            nc.tensor.matmul(
                                g_ps[c : c + CS, :],
                                wg_sb[:, n, c : c + CS],
                                x_sb[:, n, :],
                                start=(n == 0), stop=(n == kt - 1),
                                tile_position=(0, c),
                            )
                    for n in range(kt):
                        for c in range(0, P, CS):
                            nc.tensor.matmul(
                                u_ps[c : c + CS, :],
                                wu_sb[:, n, c : c + CS],
                                x_sb[:, n, :],
                                start=(n == 0), stop=(n == kt - 1),
                                tile_position=(0, c),
                            )
                    s_sb = epool.tile([P, blk], f32, tag="silu")
                    nc.scalar.activation(
                        s_sb[:], g_ps[:], _af(AF), scale=ag_sb[:, i : i + 1]
                    )
                    h_sb = hpool.tile([P, blk], bf16, tag="h")
                    nc.vector.scalar_tensor_tensor(
                        h_sb[:], u_ps[:], au_sb[:, i : i + 1], s_sb[:],
                        OP.mult, OP.mult,
                    )
                    h_tiles.append(h_sb)

                for o in range(ht):
                    o_ps = pso.tile([P, blk], f32, tag="o")
                    for n0 in range(0, it, half):
                        cnt = min(half, it - n0)
                        wd_sb = wdpool.tile([P, half, P], wdt, tag="wd")
                        nc.sync.dma_start(
                            wd_sb[:, :cnt, :], wdp[o, :, n0 : n0 + cnt, :]
                        )
                        for j in range(cnt):
                            n = n0 + j
                            for c in range(0, P, CS):
                                nc.tensor.matmul(
                                    o_ps[c : c + CS, :],
                                    wd_sb[:, j, c : c + CS],
                                    h_tiles[n][:],
                                    start=(n == 0), stop=(n == it - 1),
                                    tile_position=(0, c),
                                )
                    o_sb = opool.tile([P, blk], f32, tag="osb")
                    nc.vector.tensor_scalar_mul(o_sb[:], o_ps[:], ad_sb[:, o : o + 1])
                    nc.sync.dma_start(outp[b, o], o_sb[:])


def dedupe_ldweights(nc):
    """Post-compile BIR pass: drop InstLdweights whose stationary AP equals the
    previous PE weight load with only matmuls in between (the PE array still
    holds those weights).  Conservative: only wait-free, update-free LDWs are
    dropped.  The MMUL ISA op does not self-load, so each dropped BIR LDW is a
    real skipped LDWEIGHTS on hardware."""
    n_drop = 0
    for fn in nc.m.functions:
        for blk in fn.blocks:
            insts = list(blk.instructions)
            out = []
            last_key = None
            changed = False
            for inst in insts:
                tn = type(inst).__name__
                if tn == "InstLdweights":
                    si = inst.sync_info
                    clean = not (si and (list(si.on_wait) or list(si.on_update)))
                    try:
                        key = inst.ins[0].concise()
                    except Exception:
                        key = None
                    if clean and key is not None and key == last_key:
                        n_drop += 1
                        changed = True
                        continue
                    last_key = key
                elif tn == "InstMatmult":
                    pass  # matmuls don't clobber loaded weights
                elif getattr(inst, "engine", None) == mybir.EngineType.PE and tn not in (
                    "InstEventSemaphore",
                    "InstDrain",
                ):
                    last_key = None  # unknown PE op: be safe
                out.append(inst)
            if changed:
                blk.instructions = out
    return n_drop


def build_nc_v3(t_loc, hid, inter, blk=BLK, reps=1):
    """Paired-block variant: gate/up matmuls for both 512-token half-blocks
    share one weight load (LDW dedup), h for block B round-trips through DRAM
    so SBUF only ever holds one block's h."""
    nblk = t_loc // blk
    assert nblk == 2, "v3 pairs exactly two half-blocks"
    kt = hid // P
    it = inter // P
    ht = hid // P
    bf16 = mybir.dt.bfloat16
    f32 = mybir.dt.float32
    wdt = mybir.dt.float8e4 if WDT == "f8" else bf16
    AF = mybir.ActivationFunctionType
    OP = mybir.AluOpType

    nc = bacc.Bacc(
        "TRN2", target_bir_lowering=False, debug=False, num_devices=NCORES
    )
    xp = nc.declare_dram_parameter("xp", [nblk, P, kt, blk], bf16, isOutput=False)
    wgp = nc.declare_dram_parameter("wgp", [it, P, kt, P], wdt, isOutput=False)
    wup = nc.declare_dram_parameter("wup", [it, P, kt, P], wdt, isOutput=False)
    wdp = nc.declare_dram_parameter("wdp", [ht, P, it, P], wdt, isOutput=False)
    ags = nc.declare_dram_parameter("ags", [P, it], f32, isOutput=False)
    aus = nc.declare_dram_parameter("aus", [P, it], f32, isOutput=False)
    ads = nc.declare_dram_parameter("ads", [P, ht], f32, isOutput=False)
    outp = nc.declare_dram_parameter("outp", [nblk, ht, P, blk], f32, isOutput=True)
    hbd = nc.dram_tensor("hbd", [it, P, blk], bf16, kind="Internal")

    half = (it + 1) // 2

    with tile.TileContext(nc) as tc:
        with (
            tc.tile_pool(name="consts", bufs=1) as cpool,
            tc.tile_pool(name="xpool", bufs=1) as xpool,
            tc.tile_pool(name="wpool", bufs=2) as wpool,
            tc.tile_pool(name="wdpool", bufs=2) as wdpool,
            tc.tile_pool(name="hpool", bufs=it) as hpool,
            tc.tile_pool(name="hbpool", bufs=3) as hbpool,
            tc.tile_pool(name="epool", bufs=2) as epool,
            tc.tile_pool(name="opool", bufs=2) as opool,
            tc.tile_pool(name="psga", bufs=2, space=bass.MemorySpace.PSUM) as psga,
            tc.tile_pool(name="psgb", bufs=2, space=bass.MemorySpace.PSUM) as psgb,
            tc.tile_pool(name="psua", bufs=1, space=bass.MemorySpace.PSUM) as psua,
            tc.tile_pool(name="psub", bufs=1, space=bass.MemorySpace.PSUM) as psub,
            tc.tile_pool(name="pso", bufs=2, space=bass.MemorySpace.PSUM) as pso,
        ):
            ag_sb = cpool.tile([P, it], f32, tag="ag")
            au_sb = cpool.tile([P, it], f32, tag="au")
            ad_sb = cpool.tile([P, ht], f32, tag="ad")
            nc.sync.dma_start(ag_sb[:], ags[:])
            nc.sync.dma_start(au_sb[:], aus[:])
            nc.sync.dma_start(ad_sb[:], ads[:])

            def body():
                xa = xpool.tile([P, kt, blk], bf16, tag="xa")
                xb = xpool.tile([P, kt, blk], bf16, tag="xb")
                nc.sync.dma_start(xa[:], xp[0])
                nc.sync.dma_start(xb[:], xp[1])

                h_tiles = []
                for i in range(it):
                    wg_sb = wpool.tile([P, kt, P], wdt, tag="wg")
                    wu_sb = wpool.tile([P, kt, P], wdt, tag="wu")
                    nc.sync.dma_start(wg_sb[:], wgp[i])
                    nc.sync.dma_start(wu_sb[:], wup[i])
                    ga = psga.tile([P, blk], f32, tag="ga")
                    gb = psgb.tile([P, blk], f32, tag="gb")
                    ua = psua.tile([P, blk], f32, tag="ua")
                    ub = psub.tile([P, blk], f32, tag="ub")
                    for t in range(kt):
                        st, sp = (t == 0), (t == kt - 1)
                        nc.tensor.matmul(
                            ga[:], wg_sb[:, t, :], xa[:, t, :], start=st, stop=sp
                        )
                        nc.tensor.matmul(
                            gb[:], wg_sb[:, t, :], xb[:, t, :], start=st, stop=sp
                        )
                    for t in range(kt):
                        st, sp = (t == 0), (t == kt - 1)
                        nc.tensor.matmul(
                            ua[:], wu_sb[:, t, :], xa[:, t, :], start=st, stop=sp
                        )
                        nc.tensor.matmul(
                            ub[:], wu_sb[:, t, :], xb[:, t, :], start=st, stop=sp
                        )
                    sa = epool.tile([P, blk], f32, tag="s")
                    nc.scalar.activation(sa[:], ga[:], _af(AF), scale=ag_sb[:, i : i + 1])
                    ha = hpool.tile([P, blk], bf16, tag="h")
                    nc.vector.scalar_tensor_tensor(
                        ha[:], ua[:], au_sb[:, i : i + 1], sa[:], OP.mult, OP.mult
                    )
                    h_tiles.append(ha)
                    sb_ = epool.tile([P, blk], f32, tag="s")
                    nc.scalar.activation(
                        sb_[:], gb[:], _af(AF), scale=ag_sb[:, i : i + 1]
                    )
                    hb = hbpool.tile([P, blk], bf16, tag="hb")
                    nc.vector.scalar_tensor_tensor(
                        hb[:], ub[:], au_sb[:, i : i + 1], sb_[:], OP.mult, OP.mult
                    )
                    nc.sync.dma_start(hbd[i], hb[:])

                def down_phase(b, h_list):
                    for o in range(ht):
                        o_ps = pso.tile([P, blk], f32, tag="o")
                        for n0 in range(0, it, half):
                            cnt = min(half, it - n0)
                            wd_sb = wdpool.tile([P, half, P], wdt, tag="wd")
                            nc.sync.dma_start(
                                wd_sb[:, :cnt, :], wdp[o, :, n0 : n0 + cnt, :]
                            )
                            for j in range(cnt):
                                n = n0 + j
                                nc.tensor.matmul(
                                    o_ps[:],
                                    wd_sb[:, j, :],
                                    h_list[n][:],
                                    start=(n == 0),
                                    stop=(n == it - 1),
                                )
                        o_sb = opool.tile([P, blk], f32, tag="osb")
                        nc.vector.tensor_scalar_mul(
                            o_sb[:], o_ps[:], ad_sb[:, o : o + 1]
                        )
                        nc.sync.dma_start(outp[b, o], o_sb[:])

                down_phase(0, h_tiles)
                hb_tiles = []
                for i in range(it):
                    t = hpool.tile([P, blk], bf16, tag="h")
                    nc.sync.dma_start(t[:], hbd[i])
                    hb_tiles.append(t)
                down_phase(1, hb_tiles)

            if reps == 1:
                body()
            else:
                with tc.For_i(0, reps, 1):
                    body()
    nc.compile()
    n = dedupe_ldweights(nc)
    assert n > 0, "LDW dedup removed nothing — pairing is not effective"
    return nc


def _pack_weight(w, out_tiles, in_tiles):
    # w: [out, in] fp32 -> [out_tile, p_in, n_in, out_col] where
    # packed[i, p, n, ii] = w[i*128+ii, n*128+p]
    o, i = w.shape
    dt = ml_dtypes.float8_e4m3 if WDT == "f8" else ml_dtypes.bfloat16
    return np.ascontiguousarray(
        w.reshape(out_tiles, P, in_tiles, P).transpose(0, 3, 2, 1)
    ).astype(dt)


def _pack_scale(a, tiles):
    # a: [dim] fp32 -> [P, tiles] with packed[p, i] = a[i*128+p]
    return np.ascontiguousarray(a.reshape(tiles, P).T).astype(np.float32)


def prep_inputs(x, Wg, Wu, Wd, ag, au, ad, n_cores=NCORES, blk=BLK):
    """Host-side shard + layout prep. Returns in_maps for run_bass_kernel_spmd."""
    t = x.shape[0] * x.shape[1]
    hid = x.shape[2]
    inter = Wg.shape[0]
    t_loc = t // n_cores
    nblk = t_loc // blk
    kt = hid // P
    it = inter // P
    ht = hid // P

    wgp = _pack_weight(np.asarray(Wg), it, kt)
    wup = _pack_weight(np.asarray(Wu), it, kt)
    wdp = _pack_weight(np.asarray(Wd), ht, it)
    ags = _pack_scale(np.asarray(ag), it)
    aus = _pack_scale(np.asarray(au), it)
    ads = _pack_scale(np.asarray(ad), ht)

    xf = np.asarray(x).reshape(t, hid)
    in_maps = []
    for c in range(n_cores):
        shard = xf[c * t_loc : (c + 1) * t_loc]
        xp = np.ascontiguousarray(
            shard.reshape(nblk, blk, kt, P).transpose(0, 3, 2, 1)
        ).astype(ml_dtypes.bfloat16)
        in_maps.append(
            {"xp": xp, "wgp": wgp, "wup": wup, "wdp": wdp,
             "ags": ags, "aus": aus, "ads": ads}
        )
    return in_maps


def assemble_output(results, b=B, s=S, hid=HID, n_cores=NCORES):
    # per-core outp: [nblk, ht, P, blk] f32 -> [t_loc, hid]
    shards = []
    for c in range(n_cores):
        r = np.asarray(results[c]["outp"])
        nblk, ht, _, blk = r.shape
        shards.append(
            r.transpose(0, 3, 1, 2).reshape(nblk * blk, ht * P)
        )
    out = np.concatenate(shards, axis=0)
    return out.reshape(b, s, hid).astype(np.float32)


_NC_CACHE = {}

def kernel(x, Wg, Wu, Wd, ag, au, ad):
    t = x.shape[0] * x.shape[1]
    t_loc = t // NCORES
    key = (t, x.shape[2], Wg.shape[0])
    if key not in _NC_CACHE:
        _NC_CACHE[key] = build_nc(t_loc, x.shape[2], Wg.shape[0])
    nc = _NC_CACHE[key]
    in_maps = prep_inputs(x, Wg, Wu, Wd, ag, au, ad)
    res = run_bass_kernel_spmd(nc, in_maps, core_ids=list(range(NCORES)))
    return assemble_output(res.results, b=x.shape[0], s=x.shape[1], hid=x.shape[2])



# revision 18
# speedup vs baseline: 47.4850x; 47.4850x over previous
"""LlamaTernaryMLP (SwiGLU MLP with ternary weights + per-channel scales) on 8 TRN2 cores.

Strategy: pure data-parallel over the 8192 tokens (1024 tokens/core, no
collectives).  Each core runs the full MLP on its token shard with all three
weight matrices streamed from HBM in bf16 (ternary values are exact in bf16;
only the activations lose precision, fp32 accumulation in PSUM).

Layout (host-prepped so every DMA is >=1KB-contiguous per partition):
  activations are kept feature-major on chip ([feature,token]); weights are
  pre-transposed/tiled so each matmul is lhsT=[K=128, M=128] stationary weight
  tile against a moving [K=128, N=512] activation tile.

Per core, per 512-token block:
  gate/up: for each of 86 I-tiles accumulate 32 K-tiles over HIDDEN into two
  PSUM banks, then h = silu(gate*ag) * (up*au) -> bf16 SBUF (86 tiles resident).
  down:    for each of 32 H-tiles accumulate 86 K-tiles over INTER, scale by ad,
  DMA out fp32.
"""

import numpy as np
import ml_dtypes

import concourse.bass as bass
import concourse.mybir as mybir
import concourse.tile as tile
from concourse import bacc
from concourse.bass_utils import run_bass_kernel_spmd

P = 128
B, S, HID, INT = 4, 2048, 4096, 11008
NCORES = 8
BLK = 512
CS = 128  # matmul column-strip width (128 = no column tiling; 32/64 tested slower)
# Weight dtype: ternary {-1,0,+1} is exact in fp8e4m3.  fp8 stationary halves
# the FWL weight-load time (4 values per 32-bit read vs 2 for bf16) and the
# weight DMA traffic; the moving activations stay bf16 (mixed-dtype matmul).
WDT = "bf16"  # "f8" | "bf16" — f8 stationary measured 5x SLOWER on HW (mixed-
# dtype matmul falls off the fast path); bf16 it is.
SIM_AF = None  # set to "sigmoid" in CoreSim tests (Silu not implemented there)


def _af(AF):
    return AF.Sigmoid if SIM_AF == "sigmoid" else AF.Silu


def build_nc(t_loc, hid, inter, blk=BLK, reps=1):
    """Build the per-core Bass program for a t_loc-token shard.

    reps>1 wraps the whole computation in a hardware For_i loop (identical
    compute each iteration) — benchmarking only, so per-iteration time can be
    extracted from wall-clock above the axon RPC floor.
    """
    nblk = t_loc // blk
    kt = hid // P    # K-tiles over hidden (gate/up contraction)
    it = inter // P  # I-tiles (intermediate channels / down contraction)
    ht = hid // P    # output H-tiles
    bf16 = mybir.dt.bfloat16
    f32 = mybir.dt.float32
    wdt = mybir.dt.float8e4 if WDT == "f8" else bf16
    AF = mybir.ActivationFunctionType
    OP = mybir.AluOpType

    nc = bacc.Bacc(
        "TRN2", target_bir_lowering=False, debug=False, num_devices=NCORES
    )
    xp = nc.declare_dram_parameter("xp", [nblk, P, kt, blk], bf16, isOutput=False)
    wgp = nc.declare_dram_parameter("wgp", [it, P, kt, P], wdt, isOutput=False)
    wup = nc.declare_dram_parameter("wup", [it, P, kt, P], wdt, isOutput=False)
    wdp = nc.declare_dram_parameter("wdp", [ht, P, it, P], wdt, isOutput=False)
    ags = nc.declare_dram_parameter("ags", [P, it], f32, isOutput=False)
    aus = nc.declare_dram_parameter("aus", [P, it], f32, isOutput=False)
    ads = nc.declare_dram_parameter("ads", [P, ht], f32, isOutput=False)
    outp = nc.declare_dram_parameter("outp", [nblk, ht, P, blk], f32, isOutput=True)

    half = (it + 1) // 2  # down-proj weight strips stream in two halves

    with tile.TileContext(nc) as tc:
        with (
            tc.tile_pool(name="consts", bufs=1) as cpool,
            tc.tile_pool(name="xpool", bufs=1) as xpool,
            tc.tile_pool(name="wpool", bufs=2) as wpool,
            tc.tile_pool(name="wdpool", bufs=2) as wdpool,
            tc.tile_pool(name="hpool", bufs=it) as hpool,
            tc.tile_pool(name="epool", bufs=2) as epool,
            tc.tile_pool(name="opool", bufs=2) as opool,
            tc.tile_pool(name="psg", bufs=2, space=bass.MemorySpace.PSUM) as psg,
            tc.tile_pool(name="psu", bufs=2, space=bass.MemorySpace.PSUM) as psu,
            tc.tile_pool(name="pso", bufs=2, space=bass.MemorySpace.PSUM) as pso,
        ):
            ag_sb = cpool.tile([P, it], f32, tag="ag")
            au_sb = cpool.tile([P, it], f32, tag="au")
            ad_sb = cpool.tile([P, ht], f32, tag="ad")
            nc.sync.dma_start(ag_sb[:], ags[:])
            nc.sync.dma_start(au_sb[:], aus[:])
            nc.sync.dma_start(ad_sb[:], ads[:])

            def body():
                _build_body(
                    nc, tc, nblk, kt, it, ht, blk,
                    xp, wgp, wup, wdp, outp,
                    ag_sb, au_sb, ad_sb,
                    xpool, wpool, wdpool, hpool, epool, opool, psg, psu, pso,
                    half, bf16, f32, AF, OP, wdt,
                )

            if reps == 1:
                body()
            else:
                with tc.For_i(0, reps, 1):
                    body()
    nc.compile()
    return nc


def _build_body(
    nc, tc, nblk, kt, it, ht, blk,
    xp, wgp, wup, wdp, outp,
    ag_sb, au_sb, ad_sb,
    xpool, wpool, wdpool, hpool, epool, opool, psg, psu, pso,
    half, bf16, f32, AF, OP, wdt,
):
    for b in range(nblk):
                x_sb = xpool.tile([P, kt, blk], bf16, tag="x")
                nc.sync.dma_start(x_sb[:], xp[b])

                h_tiles = []
                for i in range(it):
                    wg_sb = wpool.tile([P, kt, P], wdt, tag="wg")
                    wu_sb = wpool.tile([P, kt, P], wdt, tag="wu")
                    nc.sync.dma_start(wg_sb[:], wgp[i])
                    nc.sync.dma_start(wu_sb[:], wup[i])
                    g_ps = psg.tile([P, blk], f32, tag="g")
                    u_ps = psu.tile([P, blk], f32, tag="u")
                    # CS=128: plain full-array matmuls. Column tiling
                    # (CS=32/64) was measured slower — the per-weight-change
                    # serialization is the array drain, which strips also pay.
                    for n in range(kt):
                        for c in range(0, P, CS):
                            nc.tensor.matmul(
                                g_ps[c : c + CS, :],
                                wg_sb[:, n, c : c + CS],
                                x_sb[:, n, :],
                                start=(n == 0), stop=(n == kt - 1),
                                tile_position=(0, c),
                            )
                    for n in range(kt):
                        for c in range(0, P, CS):
                            nc.tensor.matmul(
                                u_ps[c : c + CS, :],
                                wu_sb[:, n, c : c + CS],
                                x_sb[:, n, :],
                                start=(n == 0), stop=(n == kt - 1),
                                tile_position=(0, c),
                            )
                    s_sb = epool.tile([P, blk], f32, tag="silu")
                    nc.scalar.activation(
                        s_sb[:], g_ps[:], _af(AF), scale=ag_sb[:, i : i + 1]
                    )
                    h_sb = hpool.tile([P, blk], bf16, tag="h")
                    nc.vector.scalar_tensor_tensor(
                        h_sb[:], u_ps[:], au_sb[:, i : i + 1], s_sb[:],
                        OP.mult, OP.mult,
                    )
                    h_tiles.append(h_sb)

                for o in range(ht):
                    o_ps = pso.tile([P, blk], f32, tag="o")
                    for n0 in range(0, it, half):
                        cnt = min(half, it - n0)
                        wd_sb = wdpool.tile([P, half, P], wdt, tag="wd")
                        nc.sync.dma_start(
                            wd_sb[:, :cnt, :], wdp[o, :, n0 : n0 + cnt, :]
                        )
                        for j in range(cnt):
                            n = n0 + j
                            for c in range(0, P, CS):
                                nc.tensor.matmul(
                                    o_ps[c : c + CS, :],
                                    wd_sb[:, j, c : c + CS],
                                    h_tiles[n][:],
                                    start=(n == 0), stop=(n == it - 1),
                                    tile_position=(0, c),
                                )
                    o_sb = opool.tile([P, blk], f32, tag="osb")
                    nc.vector.tensor_scalar_mul(o_sb[:], o_ps[:], ad_sb[:, o : o + 1])
                    nc.sync.dma_start(outp[b, o], o_sb[:])


def dedupe_ldweights(nc):
    """Post-compile BIR pass: drop InstLdweights whose stationary AP equals the
    previous PE weight load with only matmuls in between (the PE array still
    holds those weights).  A dropped LDW's waits/updates move to the next
    InstMatmult, which preserves ordering (PE queue is in-order).  The MMUL ISA
    op does not self-load, so each dropped BIR LDW is a real skipped LDWEIGHTS
    on hardware."""
    n_drop = 0
    PE = mybir.EngineType.PE

    def syncs(inst):
        si = inst.sync_info
        return (list(si.on_wait), list(si.on_update)) if si else ([], [])

    for fn in nc.m.functions:
        for blk in fn.blocks:
            insts = list(blk.instructions)
            drop = set()
            last_key = None
            for idx, inst in enumerate(insts):
                tn = type(inst).__name__
                if tn == "InstLdweights":
                    try:
                        key = inst.ins[0].concise()
                    except Exception:
                        key = None
                    if key is not None and key == last_key:
                        # next PE instruction must be a matmul with room for
                        # this LDW's syncs (ISA: <=1 wait, <=1 update each)
                        nxt = None
                        for j in range(idx + 1, len(insts)):
                            if getattr(insts[j], "engine", None) == PE:
                                nxt = insts[j]
                                break
                        if nxt is not None and type(nxt).__name__ == "InstMatmult":
                            lw, lu = syncs(inst)
                            mw, mu = syncs(nxt)
                            if len(lw) + len(mw) <= 1 and len(lu) + len(mu) <= 1:
                                if lw or lu:
                                    nxt.sync_info = mybir.SyncInfo(
                                        on_wait=lw + mw, on_update=mu + lu
                                    )
                                drop.add(idx)
                                n_drop += 1
                                continue
                        # can't drop: falls through, stays the loaded key
                    last_key = key
                elif tn == "InstMatmult":
                    pass  # matmuls don't clobber loaded weights
                elif getattr(inst, "engine", None) == PE and tn not in (
                    "InstEventSemaphore",
                    "InstDrain",
                ):
                    last_key = None  # unknown PE op: be safe
            if drop:
                blk.instructions = [
                    inst for idx, inst in enumerate(insts) if idx not in drop
                ]
    return n_drop


def build_nc_v3(t_loc, hid, inter, blk=BLK, reps=1):
    """Paired-block variant: gate/up matmuls for both 512-token half-blocks
    share one weight load (LDW dedup), h for block B round-trips through DRAM
    so SBUF only ever holds one block's h."""
    nblk = t_loc // blk
    assert nblk == 2, "v3 pairs exactly two half-blocks"
    kt = hid // P
    it = inter // P
    ht = hid // P
    bf16 = mybir.dt.bfloat16
    f32 = mybir.dt.float32
    wdt = mybir.dt.float8e4 if WDT == "f8" else bf16
    AF = mybir.ActivationFunctionType
    OP = mybir.AluOpType

    nc = bacc.Bacc(
        "TRN2", target_bir_lowering=False, debug=False, num_devices=NCORES
    )
    xp = nc.declare_dram_parameter("xp", [nblk, P, kt, blk], bf16, isOutput=False)
    wgp = nc.declare_dram_parameter("wgp", [it, P, kt, P], wdt, isOutput=False)
    wup = nc.declare_dram_parameter("wup", [it, P, kt, P], wdt, isOutput=False)
    wdp = nc.declare_dram_parameter("wdp", [ht, P, it, P], wdt, isOutput=False)
    ags = nc.declare_dram_parameter("ags", [P, it], f32, isOutput=False)
    aus = nc.declare_dram_parameter("aus", [P, it], f32, isOutput=False)
    ads = nc.declare_dram_parameter("ads", [P, ht], f32, isOutput=False)
    outp = nc.declare_dram_parameter("outp", [nblk, ht, P, blk], f32, isOutput=True)
    hbd = nc.dram_tensor("hbd", [it, P, blk], bf16, kind="Internal")

    half = (it + 1) // 2

    with tile.TileContext(nc) as tc:
        with (
            tc.tile_pool(name="consts", bufs=1) as cpool,
            tc.tile_pool(name="xpool", bufs=1) as xpool,
            tc.tile_pool(name="wpool", bufs=2) as wpool,
            tc.tile_pool(name="wdpool", bufs=2) as wdpool,
            tc.tile_pool(name="hpool", bufs=it) as hpool,
            tc.tile_pool(name="hbpool", bufs=3) as hbpool,
            tc.tile_pool(name="epool", bufs=2) as epool,
            tc.tile_pool(name="opool", bufs=2) as opool,
            tc.tile_pool(name="psga", bufs=2, space=bass.MemorySpace.PSUM) as psga,
            tc.tile_pool(name="psgb", bufs=2, space=bass.MemorySpace.PSUM) as psgb,
            tc.tile_pool(name="psua", bufs=1, space=bass.MemorySpace.PSUM) as psua,
            tc.tile_pool(name="psub", bufs=1, space=bass.MemorySpace.PSUM) as psub,
            tc.tile_pool(name="pso", bufs=2, space=bass.MemorySpace.PSUM) as pso,
        ):
            ag_sb = cpool.tile([P, it], f32, tag="ag")
            au_sb = cpool.tile([P, it], f32, tag="au")
            ad_sb = cpool.tile([P, ht], f32, tag="ad")
            nc.sync.dma_start(ag_sb[:], ags[:])
            nc.sync.dma_start(au_sb[:], aus[:])
            nc.sync.dma_start(ad_sb[:], ads[:])

            def body():
                xa = xpool.tile([P, kt, blk], bf16, tag="xa")
                xb = xpool.tile([P, kt, blk], bf16, tag="xb")
                nc.sync.dma_start(xa[:], xp[0])
                nc.sync.dma_start(xb[:], xp[1])

                h_tiles = []
                for i in range(it):
                    wg_sb = wpool.tile([P, kt, P], wdt, tag="wg")
                    wu_sb = wpool.tile([P, kt, P], wdt, tag="wu")
                    nc.sync.dma_start(wg_sb[:], wgp[i])
                    nc.sync.dma_start(wu_sb[:], wup[i])
                    ga = psga.tile([P, blk], f32, tag="ga")
                    gb = psgb.tile([P, blk], f32, tag="gb")
                    ua = psua.tile([P, blk], f32, tag="ua")
                    ub = psub.tile([P, blk], f32, tag="ub")
                    for t in range(kt):
                        st, sp = (t == 0), (t == kt - 1)
                        nc.tensor.matmul(
                            ga[:], wg_sb[:, t, :], xa[:, t, :], start=st, stop=sp
                        )
                        nc.tensor.matmul(
                            gb[:], wg_sb[:, t, :], xb[:, t, :], start=st, stop=sp
                        )
                    for t in range(kt):
                        st, sp = (t == 0), (t == kt - 1)
                        nc.tensor.matmul(
                            ua[:], wu_sb[:, t, :], xa[:, t, :], start=st, stop=sp
                        )
                        nc.tensor.matmul(
                            ub[:], wu_sb[:, t, :], xb[:, t, :], start=st, stop=sp
                        )
                    sa = epool.tile([P, blk], f32, tag="s")
                    nc.scalar.activation(sa[:], ga[:], _af(AF), scale=ag_sb[:, i : i + 1])
                    ha = hpool.tile([P, blk], bf16, tag="h")
                    nc.vector.scalar_tensor_tensor(
                        ha[:], ua[:], au_sb[:, i : i + 1], sa[:], OP.mult, OP.mult
                    )
                    h_tiles.append(ha)
                    sb_ = epool.tile([P, blk], f32, tag="s")
                    nc.scalar.activation(
                        sb_[:], gb[:], _af(AF), scale=ag_sb[:, i : i + 1]
                    )
                    hb = hbpool.tile([P, blk], bf16, tag="hb")
                    nc.vector.scalar_tensor_tensor(
                        hb[:], ub[:], au_sb[:, i : i + 1], sb_[:], OP.mult, OP.mult
                    )
                    nc.sync.dma_start(hbd[i], hb[:])

                def down_phase(b, h_list):
                    for o in range(ht):
                        o_ps = pso.tile([P, blk], f32, tag="o")
                        for n0 in range(0, it, half):
                            cnt = min(half, it - n0)
                            wd_sb = wdpool.tile([P, half, P], wdt, tag="wd")
                            nc.sync.dma_start(
                                wd_sb[:, :cnt, :], wdp[o, :, n0 : n0 + cnt, :]
                            )
                            for j in range(cnt):
                                n = n0 + j
                                nc.tensor.matmul(
                                    o_ps[:],
                                    wd_sb[:, j, :],
                                    h_list[n][:],
                                    start=(n == 0),
                                    stop=(n == it - 1),
                                )
                        o_sb = opool.tile([P, blk], f32, tag="osb")
                        nc.vector.tensor_scalar_mul(
                            o_sb[:], o_ps[:], ad_sb[:, o : o + 1]
                        )
                        nc.sync.dma_start(outp[b, o], o_sb[:])

                down_phase(0, h_tiles)
                hb_tiles = []
                for i in range(it):
                    t = hpool.tile([P, blk], bf16, tag="h")
                    nc.sync.dma_start(t[:], hbd[i])
                    hb_tiles.append(t)
                down_phase(1, hb_tiles)

            if reps == 1:
                body()
            else:
                with tc.For_i(0, reps, 1):
                    body()
    nc.compile()
    n = dedupe_ldweights(nc)
    assert n > 0, "LDW dedup removed nothing — pairing is not effective"
    return nc


def build_nc_v5(t_loc, hid, inter, blk=BLK, reps=1):
    """Both phases paired: every weight tile (stationary) is loaded once and
    used by two matmuls (one per 512-token half-block), halving LDWEIGHTS
    overhead.  h for BOTH blocks round-trips through DRAM; phase 2 streams it
    back per o-tile-pair pass.  PSUM: 4 tags x bufs=2 = 8 banks shared by both
    phases."""
    nblk = t_loc // blk
    assert nblk == 2
    kt = hid // P
    it = inter // P
    ht = hid // P
    bf16 = mybir.dt.bfloat16
    f32 = mybir.dt.float32
    wdt = mybir.dt.float8e4 if WDT == "f8" else bf16
    AF = mybir.ActivationFunctionType
    OP = mybir.AluOpType

    nc = bacc.Bacc(
        "TRN2", target_bir_lowering=False, debug=False, num_devices=NCORES
    )
    xp = nc.declare_dram_parameter("xp", [nblk, P, kt, blk], bf16, isOutput=False)
    wgp = nc.declare_dram_parameter("wgp", [it, P, kt, P], wdt, isOutput=False)
    wup = nc.declare_dram_parameter("wup", [it, P, kt, P], wdt, isOutput=False)
    wdp = nc.declare_dram_parameter("wdp", [ht, P, it, P], wdt, isOutput=False)
    ags = nc.declare_dram_parameter("ags", [P, it], f32, isOutput=False)
    aus = nc.declare_dram_parameter("aus", [P, it], f32, isOutput=False)
    ads = nc.declare_dram_parameter("ads", [P, ht], f32, isOutput=False)
    outp = nc.declare_dram_parameter("outp", [nblk, ht, P, blk], f32, isOutput=True)
    hd = nc.dram_tensor("hd", [nblk, it, P, blk], bf16, kind="Internal")

    half = (it + 1) // 2

    with tile.TileContext(nc) as tc:
        with (
            tc.tile_pool(name="consts", bufs=1) as cpool,
            tc.tile_pool(name="xpool", bufs=1) as xpool,
            tc.tile_pool(name="wpool", bufs=2) as wpool,
            tc.tile_pool(name="wdpool", bufs=4) as wdpool,
            tc.tile_pool(name="hspool", bufs=16) as hspool,
            tc.tile_pool(name="hopool", bufs=4) as hopool,
            tc.tile_pool(name="epool", bufs=2) as epool,
            tc.tile_pool(name="opool", bufs=4) as opool,
            tc.tile_pool(name="ps", bufs=2, space=bass.MemorySpace.PSUM) as ps,
        ):
            ag_sb = cpool.tile([P, it], f32, tag="ag")
            au_sb = cpool.tile([P, it], f32, tag="au")
            ad_sb = cpool.tile([P, ht], f32, tag="ad")
            nc.sync.dma_start(ag_sb[:], ags[:])
            nc.sync.dma_start(au_sb[:], aus[:])
            nc.sync.dma_start(ad_sb[:], ads[:])

            def body():
                xa = xpool.tile([P, kt, blk], bf16, tag="xa")
                xb = xpool.tile([P, kt, blk], bf16, tag="xb")
                nc.sync.dma_start(xa[:], xp[0])
                nc.sync.dma_start(xb[:], xp[1])

                # ---- phase 1: gate/up, both blocks per weight tile ----
                for i in range(it):
                    wg_sb = wpool.tile([P, kt, P], wdt, tag="wg")
                    wu_sb = wpool.tile([P, kt, P], wdt, tag="wu")
                    nc.sync.dma_start(wg_sb[:], wgp[i])
                    nc.sync.dma_start(wu_sb[:], wup[i])
                    ga = ps.tile([P, blk], f32, tag="p0")
                    gb = ps.tile([P, blk], f32, tag="p1")
                    ua = ps.tile([P, blk], f32, tag="p2")
                    ub = ps.tile([P, blk], f32, tag="p3")
                    for t in range(kt):
                        st, sp = (t == 0), (t == kt - 1)
                        nc.tensor.matmul(
                            ga[:], wg_sb[:, t, :], xa[:, t, :], start=st, stop=sp
                        )
                        nc.tensor.matmul(
                            gb[:], wg_sb[:, t, :], xb[:, t, :], start=st, stop=sp
                        )
                    for t in range(kt):
                        st, sp = (t == 0), (t == kt - 1)
                        nc.tensor.matmul(
                            ua[:], wu_sb[:, t, :], xa[:, t, :], start=st, stop=sp
                        )
                        nc.tensor.matmul(
                            ub[:], wu_sb[:, t, :], xb[:, t, :], start=st, stop=sp
                        )
                    for b, g_ps, u_ps in ((0, ga, ua), (1, gb, ub)):
                        s_sb = epool.tile([P, blk], f32, tag="s")
                        nc.scalar.activation(
                            s_sb[:], g_ps[:], _af(AF), scale=ag_sb[:, i : i + 1]
                        )
                        h_sb = hopool.tile([P, blk], bf16, tag="ho")
                        nc.vector.scalar_tensor_tensor(
                            h_sb[:], u_ps[:], au_sb[:, i : i + 1], s_sb[:],
                            OP.mult, OP.mult,
                        )
                        nc.sync.dma_start(hd[b, i], h_sb[:])

                # ---- phase 2: down-proj, o-tiles in pairs, h streamed ----
                for og in range(0, ht, 2):
                    acc0 = ps.tile([P, blk], f32, tag="p0")
                    acc1 = ps.tile([P, blk], f32, tag="p1")
                    acc2 = ps.tile([P, blk], f32, tag="p2")
                    acc3 = ps.tile([P, blk], f32, tag="p3")
                    acc = [acc0, acc1, acc2, acc3]
                    for n0 in range(0, it, half):
                        cnt = min(half, it - n0)
                        wd0 = wdpool.tile([P, half, P], wdt, tag="wd")
                        wd1 = wdpool.tile([P, half, P], wdt, tag="wd")
                        nc.sync.dma_start(wd0[:, :cnt, :], wdp[og, :, n0 : n0 + cnt, :])
                        nc.sync.dma_start(
                            wd1[:, :cnt, :], wdp[og + 1, :, n0 : n0 + cnt, :]
                        )
                        for j in range(cnt):
                            n = n0 + j
                            ha = hspool.tile([P, blk], bf16, tag="hs")
                            hb = hspool.tile([P, blk], bf16, tag="hs")
                            nc.sync.dma_start(ha[:], hd[0, n])
                            nc.sync.dma_start(hb[:], hd[1, n])
                            st, sp = (n == 0), (n == it - 1)
                            nc.tensor.matmul(
                                acc[0][:], wd0[:, j, :], ha[:], start=st, stop=sp
                            )
                            nc.tensor.matmul(
                                acc[1][:], wd0[:, j, :], hb[:], start=st, stop=sp
                            )
                            nc.tensor.matmul(
                                acc[2][:], wd1[:, j, :], ha[:], start=st, stop=sp
                            )
                            nc.tensor.matmul(
                                acc[3][:], wd1[:, j, :], hb[:], start=st, stop=sp
                            )
                    for q, (o, b) in enumerate(
                        ((og, 0), (og, 1), (og + 1, 0), (og + 1, 1))
                    ):
                        o_sb = opool.tile([P, blk], f32, tag="osb")
                        nc.vector.tensor_scalar_mul(
                            o_sb[:], acc[q][:], ad_sb[:, o : o + 1]
                        )
                        nc.sync.dma_start(outp[b, o], o_sb[:])

            if reps == 1:
                body()
            else:
                with tc.For_i(0, reps, 1):
                    body()
    nc.compile()
    n = dedupe_ldweights(nc)
    assert n > 0, "LDW dedup removed nothing"
    return nc


def build_nc_v6(t_loc, hid, inter, blk=BLK, reps=1):
    """v6: both phases paired across the two 512-token half-blocks (every
    stationary weight tile serves 2 matmuls -> half the LDWEIGHTS), h for both
    blocks round-trips through DRAM, phase 2 processes FOUR o-tiles per h
    streaming pass (8 PSUM accumulators) with h DMA'd in 4-tile batches."""
    nblk = t_loc // blk
    assert nblk == 2
    kt = hid // P
    it = inter // P
    ht = hid // P
    bf16 = mybir.dt.bfloat16
    f32 = mybir.dt.float32
    wdt = mybir.dt.float8e4 if WDT == "f8" else bf16
    AF = mybir.ActivationFunctionType
    OP = mybir.AluOpType
    QL = 22  # down-proj weight strip length (quarters of 86)
    HG = 4   # h-stream DMA batch (tiles per descriptor)

    nc = bacc.Bacc(
        "TRN2", target_bir_lowering=False, debug=False, num_devices=NCORES
    )
    xp = nc.declare_dram_parameter("xp", [nblk, P, kt, blk], bf16, isOutput=False)
    wgp = nc.declare_dram_parameter("wgp", [it, P, kt, P], wdt, isOutput=False)
    wup = nc.declare_dram_parameter("wup", [it, P, kt, P], wdt, isOutput=False)
    wdp = nc.declare_dram_parameter("wdp", [ht, P, it, P], wdt, isOutput=False)
    ags = nc.declare_dram_parameter("ags", [P, it], f32, isOutput=False)
    aus = nc.declare_dram_parameter("aus", [P, it], f32, isOutput=False)
    ads = nc.declare_dram_parameter("ads", [P, ht], f32, isOutput=False)
    outp = nc.declare_dram_parameter("outp", [nblk, ht, P, blk], f32, isOutput=True)
    hd = nc.dram_tensor("hd", [nblk, P, it, blk], bf16, kind="Internal")

    with tile.TileContext(nc) as tc:
        with (
            tc.tile_pool(name="consts", bufs=1) as cpool,
            tc.tile_pool(name="xpool", bufs=1) as xpool,
            tc.tile_pool(name="wpool", bufs=2) as wpool,
            tc.tile_pool(name="wdpool", bufs=8) as wdpool,
            tc.tile_pool(name="hspool", bufs=6) as hspool,
            tc.tile_pool(name="hopool", bufs=4) as hopool,
            tc.tile_pool(name="epool", bufs=2) as epool,
            tc.tile_pool(name="opool", bufs=4) as opool,
            tc.tile_pool(name="ps", bufs=1, space=bass.MemorySpace.PSUM) as ps,
        ):
            ag_sb = cpool.tile([P, it], f32, tag="ag")
            au_sb = cpool.tile([P, it], f32, tag="au")
            ad_sb = cpool.tile([P, ht], f32, tag="ad")
            nc.sync.dma_start(ag_sb[:], ags[:])
            nc.sync.dma_start(au_sb[:], aus[:])
            nc.sync.dma_start(ad_sb[:], ads[:])

            def body():
                xa = xpool.tile([P, kt, blk], bf16, tag="xa")
                xb = xpool.tile([P, kt, blk], bf16, tag="xb")
                nc.sync.dma_start(xa[:], xp[0])
                nc.sync.dma_start(xb[:], xp[1])

                # ---- phase 1: gate/up, both blocks per weight tile ----
                for i in range(it):
                    wg_sb = wpool.tile([P, kt, P], wdt, tag="wg")
                    wu_sb = wpool.tile([P, kt, P], wdt, tag="wu")
                    nc.sync.dma_start(wg_sb[:], wgp[i])
                    nc.sync.dma_start(wu_sb[:], wup[i])
                    q = 4 * (i % 2)  # alternate PSUM tag quads -> double buffer
                    ga = ps.tile([P, blk], f32, tag=f"p{q}")
                    gb = ps.tile([P, blk], f32, tag=f"p{q + 1}")
                    ua = ps.tile([P, blk], f32, tag=f"p{q + 2}")
                    ub = ps.tile([P, blk], f32, tag=f"p{q + 3}")
                    for t in range(kt):
                        st, sp = (t == 0), (t == kt - 1)
                        nc.tensor.matmul(
                            ga[:], wg_sb[:, t, :], xa[:, t, :], start=st, stop=sp
                        )
                        nc.tensor.matmul(
                            gb[:], wg_sb[:, t, :], xb[:, t, :], start=st, stop=sp
                        )
                    for t in range(kt):
                        st, sp = (t == 0), (t == kt - 1)
                        nc.tensor.matmul(
                            ua[:], wu_sb[:, t, :], xa[:, t, :], start=st, stop=sp
                        )
                        nc.tensor.matmul(
                            ub[:], wu_sb[:, t, :], xb[:, t, :], start=st, stop=sp
                        )
                    for b, g_ps, u_ps in ((0, ga, ua), (1, gb, ub)):
                        s_sb = epool.tile([P, blk], f32, tag="s")
                        nc.scalar.activation(
                            s_sb[:], g_ps[:], _af(AF), scale=ag_sb[:, i : i + 1]
                        )
                        h_sb = hopool.tile([P, blk], bf16, tag="ho")
                        nc.vector.scalar_tensor_tensor(
                            h_sb[:], u_ps[:], au_sb[:, i : i + 1], s_sb[:],
                            OP.mult, OP.mult,
                        )
                        nc.sync.dma_start(hd[b, :, i, :], h_sb[:])

                # ---- phase 2: down-proj, 4 o-tiles per h streaming pass ----
                for og in range(0, ht, 4):
                    acc0 = ps.tile([P, blk], f32, tag="p0")
                    acc1 = ps.tile([P, blk], f32, tag="p1")
                    acc2 = ps.tile([P, blk], f32, tag="p2")
                    acc3 = ps.tile([P, blk], f32, tag="p3")
                    acc4 = ps.tile([P, blk], f32, tag="p4")
                    acc5 = ps.tile([P, blk], f32, tag="p5")
                    acc6 = ps.tile([P, blk], f32, tag="p6")
                    acc7 = ps.tile([P, blk], f32, tag="p7")
                    accs = [acc0, acc1, acc2, acc3, acc4, acc5, acc6, acc7]
                    for n0 in range(0, it, QL):
                        cnt = min(QL, it - n0)
                        wds = []
                        for oo in range(4):
                            w_sb = wdpool.tile([P, QL, P], wdt, tag="wd")
                            nc.sync.dma_start(
                                w_sb[:, :cnt, :], wdp[og + oo, :, n0 : n0 + cnt, :]
                            )
                            wds.append(w_sb)
                        for g0 in range(n0, n0 + cnt, HG):
                            gc = min(HG, n0 + cnt - g0)
                            ha = hspool.tile([P, HG, blk], bf16, tag="hs")
                            hb = hspool.tile([P, HG, blk], bf16, tag="hs")
                            nc.sync.dma_start(
                                ha[:, :gc, :], hd[0, :, g0 : g0 + gc, :]
                            )
                            nc.sync.dma_start(
                                hb[:, :gc, :], hd[1, :, g0 : g0 + gc, :]
                            )
                            for g in range(gc):
                                n = g0 + g
                                j = n - n0
                                st, sp = (n == 0), (n == it - 1)
                                for oo in range(4):
                                    nc.tensor.matmul(
                                        accs[2 * oo][:], wds[oo][:, j, :],
                                        ha[:, g, :], start=st, stop=sp,
                                    )
                                    nc.tensor.matmul(
                                        accs[2 * oo + 1][:], wds[oo][:, j, :],
                                        hb[:, g, :], start=st, stop=sp,
                                    )
                    for oo in range(4):
                        for b in range(2):
                            o_sb = opool.tile([P, blk], f32, tag="osb")
                            nc.vector.tensor_scalar_mul(
                                o_sb[:], accs[2 * oo + b][:],
                                ad_sb[:, og + oo : og + oo + 1],
                            )
                            nc.sync.dma_start(outp[b, og + oo], o_sb[:])

            if reps == 1:
                body()
            else:
                with tc.For_i(0, reps, 1):
                    body()
    nc.compile()
    n = dedupe_ldweights(nc)
    assert n > 0, "LDW dedup removed nothing"
    return nc


def _pack_weight(w, out_tiles, in_tiles):
    # w: [out, in] fp32 -> [out_tile, p_in, n_in, out_col] where
    # packed[i, p, n, ii] = w[i*128+ii, n*128+p]
    o, i = w.shape
    dt = ml_dtypes.float8_e4m3 if WDT == "f8" else ml_dtypes.bfloat16
    return np.ascontiguousarray(
        w.reshape(out_tiles, P, in_tiles, P).transpose(0, 3, 2, 1)
    ).astype(dt)


def _pack_scale(a, tiles):
    # a: [dim] fp32 -> [P, tiles] with packed[p, i] = a[i*128+p]
    return np.ascontiguousarray(a.reshape(tiles, P).T).astype(np.float32)


def prep_inputs(x, Wg, Wu, Wd, ag, au, ad, n_cores=NCORES, blk=BLK):
    """Host-side shard + layout prep. Returns in_maps for run_bass_kernel_spmd."""
    t = x.shape[0] * x.shape[1]
    hid = x.shape[2]
    inter = Wg.shape[0]
    t_loc = t // n_cores
    nblk = t_loc // blk
    kt = hid // P
    it = inter // P
    ht = hid // P

    wgp = _pack_weight(np.asarray(Wg), it, kt)
    wup = _pack_weight(np.asarray(Wu), it, kt)
    wdp = _pack_weight(np.asarray(Wd), ht, it)
    ags = _pack_scale(np.asarray(ag), it)
    aus = _pack_scale(np.asarray(au), it)
    ads = _pack_scale(np.asarray(ad), ht)

    xf = np.asarray(x).reshape(t, hid)
    in_maps = []
    for c in range(n_cores):
        shard = xf[c * t_loc : (c + 1) * t_loc]
        xp = np.ascontiguousarray(
            shard.reshape(nblk, blk, kt, P).transpose(0, 3, 2, 1)
        ).astype(ml_dtypes.bfloat16)
        in_maps.append(
            {"xp": xp, "wgp": wgp, "wup": wup, "wdp": wdp,
             "ags": ags, "aus": aus, "ads": ads}
        )
    return in_maps


def assemble_output(results, b=B, s=S, hid=HID, n_cores=NCORES):
    # per-core outp: [nblk, ht, P, blk] f32 -> [t_loc, hid]
    shards = []
    for c in range(n_cores):
        r = np.asarray(results[c]["outp"])
        nblk, ht, _, blk = r.shape
        shards.append(
            r.transpose(0, 3, 1, 2).reshape(nblk * blk, ht * P)
        )
    out = np.concatenate(shards, axis=0)
    return out.reshape(b, s, hid).astype(np.float32)


_NC_CACHE = {}

def kernel(x, Wg, Wu, Wd, ag, au, ad):
    t = x.shape[0] * x.shape[1]
    t_loc = t // NCORES
    key = (t, x.shape[2], Wg.shape[0])
    if key not in _NC_CACHE:
        _NC_CACHE[key] = build_nc(t_loc, x.shape[2], Wg.shape[0])
    nc = _NC_CACHE[key]
    in_maps = prep_inputs(x, Wg, Wu, Wd, ag, au, ad)
    res = run_bass_kernel_spmd(nc, in_maps, core_ids=list(range(NCORES)))
    return assemble_output(res.results, b=x.shape[0], s=x.shape[1], hid=x.shape[2])

